# revision 1
# baseline (speedup 1.0000x reference)
"""Contrastive loss kernel for Trainium2 (8 NeuronCores, batch-parallel).

Problem (hardcoded):
  X: (32, 16384, 256) f32   pair embeddings, e_a = X[..., :128], e_b = X[..., 128:]
  y: (32, 128, 128)  i32    adjacency in {0, 1}
  out: (32, 16384)   f32    where(y==1, dist2, relu(1 - dist2))

Sharding: data-parallel over batch, 4 batches per core, no communication.
"""

from contextlib import ExitStack

import numpy as np

import concourse.bass as bass
import concourse.tile as tile
from concourse import bacc, masks, mybir
from concourse.bass_utils import run_bass_kernel_spmd

F32 = mybir.dt.float32
I32 = mybir.dt.int32

B, P, D = 32, 16384, 256
H = D // 2  # 128
ALPHA_MARGIN = 1.0
N_CORES = 8
BPC = B // N_CORES  # batches per core

PART = 128           # SBUF partitions; also pairs per result column
SLOTS = 8            # pair-columns per big tile
TILES = P // (PART * SLOTS)  # big tiles per batch (16)


def build_program(bpc=BPC, slots=SLOTS, tiles=None, pairs=P, passes=1,
                  xbufs=3, dma_split=False):
    """Build the per-core Bass program. Shapes are per-core (full batch dim / 8).

    passes>1 repeats the whole computation (idempotent) — used only for
    marginal-time benchmarking, never for the graded kernel."""
    if tiles is None:
        tiles = pairs // (PART * slots)
    assert tiles * slots * PART == pairs
    ncols = tiles * slots  # result columns per batch (pairs // 128)

    nc = bacc.Bacc("TRN2", target_bir_lowering=False, debug=False,
                   num_devices=N_CORES)
    X = nc.dram_tensor("X", [bpc, pairs, D], F32, kind="ExternalInput").ap()
    Y = nc.dram_tensor("y", [bpc, pairs], I32, kind="ExternalInput").ap()
    O = nc.dram_tensor("out", [bpc, pairs], F32, kind="ExternalOutput").ap()

    with tile.TileContext(nc) as tc, ExitStack() as ctx:
        xpool = ctx.enter_context(tc.tile_pool(name="x", bufs=xbufs))
        dpool = ctx.enter_context(tc.tile_pool(name="diff", bufs=3))
        rpool = ctx.enter_context(tc.tile_pool(name="res", bufs=2))
        ppool = ctx.enter_context(tc.tile_pool(name="psum", bufs=2, space="PSUM"))
        spool = ctx.enter_context(tc.tile_pool(name="small", bufs=2))
        cpool = ctx.enter_context(tc.tile_pool(name="const", bufs=1))

        ident = cpool.tile([PART, PART], F32)
        masks.make_identity(nc, ident[:])
        ones = cpool.tile([PART, 1], F32)
        nc.gpsimd.memset(ones[:], 1.0)

        for b in [b for _ in range(passes) for b in range(bpc)]:
            # pair index = t*128 + p  ->  [p, t, f] view of X[b]
            Xb = X[b].rearrange("(t p) f -> p t f", p=PART)
            res = rpool.tile([PART, ncols], F32)
            for g in range(tiles):
                xt = xpool.tile([PART, slots, D], F32)
                dma_eng = nc.scalar if (dma_split and g % 2) else nc.sync
                dma_eng.dma_start(xt[:], Xb[:, g * slots:(g + 1) * slots, :])
                dft = dpool.tile([PART, slots, H], F32)
                nc.vector.tensor_sub(dft[:], xt[:, :, 0:H], xt[:, :, H:D])
                for j in range(slots):
                    c = g * slots + j
                    # DVE: out = diff * diff, accum_out = sum -> dist2.
                    # All-DVE beats splitting with ACT (modeled 197.5 vs
                    # 198.0/257.2 us): DVE stays 70 us under the DMA span
                    # while ACT's per-op overhead is 2.6x DVE's.
                    nc.vector.scalar_tensor_tensor(
                        out=dft[:, j, :], in0=dft[:, j, :], scalar=0.0,
                        in1=dft[:, j, :],
                        op0=mybir.AluOpType.bypass, op1=mybir.AluOpType.mult,
                        accum_out=res[:, c:c + 1],
                    )

            # res[p, t] = dist2(pair t*128+p); transpose so partition = t
            pres = ppool.tile([ncols, PART], F32)
            nc.tensor.transpose(pres[:], res[:], ident[:])

            yt = spool.tile([ncols, PART], I32)
            nc.sync.dma_start(yt[:], Y[b].rearrange("(t p) -> t p", p=PART))

            # outt = relu(margin - dist2); then overwrite y==1 entries with dist2
            outt = spool.tile([ncols, PART], F32)
            nc.scalar.activation(
                outt[:], pres[:], mybir.ActivationFunctionType.Relu,
                scale=-1.0, bias=ones[0:ncols, 0:1],
            )
            nc.vector.copy_predicated(outt[:], yt[:], pres[:])

            nc.sync.dma_start(O[b].rearrange("(t p) -> t p", p=PART), outt[:])

    nc.compile()
    return nc


_PROGRAM_CACHE = {}


def _get_program():
    if "nc" not in _PROGRAM_CACHE:
        _PROGRAM_CACHE["nc"] = build_program()
    return _PROGRAM_CACHE["nc"]


def kernel(X, y):
    import os
    if os.environ.get("BASS_TRACE"):
        # The axon NTFF trace path needs antenv.axon_hooks, which some
        # images lack; fall back to untraced execution rather than crash.
        try:
            import antenv.axon_hooks  # noqa: F401
        except ImportError:
            os.environ["BASS_NEVER_TRACE"] = "1"

    X = np.asarray(X, dtype=np.float32)
    y = np.asarray(y, dtype=np.int32).reshape(B, P)
    assert X.shape == (B, P, D)

    nc = _get_program()
    in_maps = [
        {"X": np.ascontiguousarray(X[c * BPC:(c + 1) * BPC]),
         "y": np.ascontiguousarray(y[c * BPC:(c + 1) * BPC])}
        for c in range(N_CORES)
    ]
    # The axon-tunneled devices occasionally come up wedged from a prior
    # session (NRT_EXEC_UNIT_UNRECOVERABLE); a backend reset + retry clears it.
    last_err = None
    for attempt in range(3):
        try:
            res = run_bass_kernel_spmd(nc, in_maps, list(range(N_CORES)))
            break
        except Exception as e:  # transient device/tunnel failures
            last_err = e
            import time

            import jax
            try:
                jax.clear_caches()
            except Exception:
                pass
            try:
                jax._src.api.clear_backends()
            except Exception:
                pass
            time.sleep(5.0 * (attempt + 1))
    else:
        raise last_err
    out = np.concatenate([res.results[c]["out"] for c in range(N_CORES)], axis=0)
    return out.astype(np.float32)



# revision 2
# speedup vs baseline: 84.2534x; 84.2534x over previous
"""Contrastive loss kernel for Trainium2 (8 NeuronCores, batch-parallel).

Problem (hardcoded):
  X: (32, 16384, 256) f32   pair embeddings, e_a = X[..., :128], e_b = X[..., 128:]
  y: (32, 128, 128)  i32    adjacency in {0, 1}
  out: (32, 16384)   f32    where(y==1, dist2, relu(1 - dist2))

Sharding: data-parallel over batch, 4 batches per core, no communication.

Pair-chunk layout: partition p owns pairs [p*128, (p+1)*128) of each batch,
so every X-tile DMA partition line is one contiguous 32 KiB HBM run
(128 descriptors of 32 KiB per 4 MiB dma_start). Measured ~5-8% faster than
the v1 interleaved layout (1 KiB descriptor lines): big descriptors amortize
per-descriptor overhead while the whole 64 MiB/core stream stays in HBM
address order. Single HWDGE queue (sync): splitting across sync+scalar rings
interleaves the address stream at packet granularity and measured slower.
The result rows land in row-major output order directly, so the v1
TensorE transpose + PSUM stage disappears.
"""

from contextlib import ExitStack

import numpy as np

import concourse.bass as bass
import concourse.tile as tile
from concourse import bacc, mybir
from concourse.bass_utils import run_bass_kernel_spmd

F32 = mybir.dt.float32
I32 = mybir.dt.int32

B, P, D = 32, 16384, 256
H = D // 2  # 128
ALPHA_MARGIN = 1.0
N_CORES = 8
BPC = B // N_CORES  # batches per core

PART = 128           # SBUF partitions
PPP = P // PART      # pairs per partition per batch = 128
C = 32               # pairs per partition per X-tile DMA (4 MiB per dma_start)


def build_program(bpc=BPC, C=C, passes=1, loop=None, xbufs=2, dbufs=2):
    """Build the per-core Bass program. Shapes are per-core (full batch / 8).

    passes/loop repeat the computation (idempotent) — benchmarking only,
    never used by the graded kernel()."""
    assert PPP % C == 0
    tiles = PPP // C

    nc = bacc.Bacc("TRN2", target_bir_lowering=False, debug=False,
                   num_devices=N_CORES)
    X = nc.dram_tensor("X", [bpc, P, D], F32, kind="ExternalInput").ap()
    Y = nc.dram_tensor("y", [bpc, P], I32, kind="ExternalInput").ap()
    O = nc.dram_tensor("out", [bpc, P], F32, kind="ExternalOutput").ap()

    with tile.TileContext(nc) as tc, ExitStack() as ctx:
        xpool = ctx.enter_context(tc.tile_pool(name="x", bufs=xbufs))
        dpool = ctx.enter_context(tc.tile_pool(name="diff", bufs=dbufs))
        rpool = ctx.enter_context(tc.tile_pool(name="res", bufs=2))
        spool = ctx.enter_context(tc.tile_pool(name="small", bufs=2))
        cpool = ctx.enter_context(tc.tile_pool(name="const", bufs=1))

        ones = cpool.tile([PART, 1], F32)
        nc.gpsimd.memset(ones[:], 1.0)

        def body():
            for b in range(bpc):
                # pair index = p*128 + j  ->  [p, j, f] view of X[b]
                Xb = X[b].rearrange("(p j) f -> p j f", p=PART)
                res = rpool.tile([PART, PPP], F32)
                for g in range(tiles):
                    xt = xpool.tile([PART, C, D], F32)
                    nc.sync.dma_start(xt[:], Xb[:, g * C:(g + 1) * C, :])
                    dft = dpool.tile([PART, C, H], F32)
                    nc.vector.tensor_sub(dft[:], xt[:, :, 0:H], xt[:, :, H:D])
                    for j in range(C):
                        c = g * C + j
                        # DVE: out = diff * diff, accum_out = sum -> dist2.
                        # DVE stays well under the DMA span; ACT per-op
                        # overhead is 2.6x DVE's, so all-DVE wins.
                        nc.vector.scalar_tensor_tensor(
                            out=dft[:, j, :], in0=dft[:, j, :], scalar=0.0,
                            in1=dft[:, j, :],
                            op0=mybir.AluOpType.bypass,
                            op1=mybir.AluOpType.mult,
                            accum_out=res[:, c:c + 1],
                        )

                yt = spool.tile([PART, PPP], I32)
                nc.sync.dma_start(yt[:], Y[b].rearrange("(p j) -> p j", p=PART))

                # outt = relu(margin - dist2); overwrite y==1 entries with dist2
                outt = spool.tile([PART, PPP], F32)
                nc.scalar.activation(
                    outt[:], res[:], mybir.ActivationFunctionType.Relu,
                    scale=-1.0, bias=ones[:, 0:1],
                )
                nc.vector.copy_predicated(outt[:], yt[:], res[:])
                nc.sync.dma_start(O[b].rearrange("(p j) -> p j", p=PART),
                                  outt[:])

        if loop is not None:
            with tc.For_i(0, loop):
                for _ in range(passes):
                    body()
        else:
            for _ in range(passes):
                body()

    nc.compile()
    return nc


_PROGRAM_CACHE = {}


def _get_program():
    if "nc" not in _PROGRAM_CACHE:
        _PROGRAM_CACHE["nc"] = build_program()
    return _PROGRAM_CACHE["nc"]


def kernel(X, y):
    import os
    if os.environ.get("BASS_TRACE"):
        # The axon NTFF trace path needs antenv.axon_hooks, which some
        # images lack; fall back to untraced execution rather than crash.
        try:
            import antenv.axon_hooks  # noqa: F401
        except ImportError:
            os.environ["BASS_NEVER_TRACE"] = "1"

    X = np.asarray(X, dtype=np.float32)
    y = np.asarray(y, dtype=np.int32).reshape(B, P)
    assert X.shape == (B, P, D)

    nc = _get_program()
    in_maps = [
        {"X": np.ascontiguousarray(X[c * BPC:(c + 1) * BPC]),
         "y": np.ascontiguousarray(y[c * BPC:(c + 1) * BPC])}
        for c in range(N_CORES)
    ]
    # The axon-tunneled devices occasionally come up wedged from a prior
    # session (NRT_EXEC_UNIT_UNRECOVERABLE); a backend reset + retry clears it.
    last_err = None
    for attempt in range(3):
        try:
            res = run_bass_kernel_spmd(nc, in_maps, list(range(N_CORES)))
            break
        except Exception as e:  # transient device/tunnel failures
            last_err = e
            import time

            import jax
            try:
                jax.clear_caches()
            except Exception:
                pass
            try:
                jax._src.api.clear_backends()
            except Exception:
                pass
            time.sleep(5.0 * (attempt + 1))
    else:
        raise last_err
    out = np.concatenate([res.results[c]["out"] for c in range(N_CORES)], axis=0)
    return out.astype(np.float32)
